# revision 53
# baseline (speedup 1.0000x reference)
"""FBPINN forward kernel for Trainium2 (8 NeuronCores, SPMD data parallel).

Strategy
--------
The reference evaluates 64 small MLPs (2->32->32->32->1, tanh) on all 65536
points and combines them with compactly-supported sigmoid windows:
    u(x) = sum_s w_s(x) y_s(x) / (sum_s w_s(x) + 1e-8)
The window w_s decays like exp(-266*d) with distance d outside subdomain s's
core cell, so pairs beyond d ~ 0.02 contribute only a few 1e-3 relative.  We
bin points into subdomains whose core cell (dilated by 0.030 / 0.022, then
trimmed to fixed caps keeping the closest points) contains them — ~1.7
subnets per point instead of the dense 64 — run the per-subdomain MLP
batches on the device, and scatter/normalize on the host (rel err ~2.1e-3
vs the 2e-2 gate).

Sharding: 8 subdomains per core (subdomain-parallel); bins are size-sorted:
the 32 largest go to half-A slots (4 full-bank chunks, <=2048 points) and
the 32 smallest to half-B (3 chunks, <=1536); all 8 cores run an identical
program (SPMD).

Device kernel (per core): 8 subnets, two halves of 4, each using
block-diagonal [128,128] float32r stationary weights (4 subnets x 32 hidden
on the partition dim; f32r streams 1 col/cycle when the moving dim >= 256).
Points stream on the free dim in full 512-col PSUM-bank chunks, so every
tanh is ONE contiguous scalar-engine read per half-layer (6 total; strided
PSUM reads >1792 elements crash the HW, contiguous ones don't).  The ACT
engine is the bottleneck (1 col/cycle @ 1.2 GHz over 3 layers), so column
count and ACT instruction count dominate.  The input layer folds its bias
via a constant 1.0 row packed into h0.  PSUM is exactly 8 banks: half-A
tile (4) + half-B tile (3) + a dedicated output accumulator (1), each pool
bufs=1 — ACT(A) frees its tile before PE needs it again, and the output
layer never blocks the next body's matmuls.  The output layer accumulates
each half's chunks into the accumulator bank with column-shifted fp16 W_out
block variants (chunk j's 4 outputs land on partitions 32j..32j+3; fp16
stationaries double-buffer their PE loads, unlike f32r), then one DVE copy
and one DMA per half.  The variants are scattered on-device (memset + 7
tiny DVE copies) from a packed [128,32] fp16 input so the weight DMA stays
small.  Timing builds (loop / reps>1) software-pipeline: body r computes
its 3 tanh layers while the OUTPUT of body r-1 (double-buffered h3) issues
during layer 1, keeping the in-order PE queue off the inter-body critical
path.  Windows, output bias/scale/shift and the final scatter-normalize are
host-side (cheap vectorized numpy).
"""

import numpy as np

import concourse.bass as bass
import concourse.tile as tile
from concourse import bacc, mybir
from concourse.bass_utils import run_bass_kernel_spmd

# ---------------------------------------------------------------- constants
N_PTS = 65536
IN_DIM = 2
HID = 32
S_TOT = 64
N_CORES = 8
SUBS_PER_CORE = 8  # 2 halves x 4 subnets
EXT = 0.0375       # reference's extended-box extension beyond the core cell
# Binning margins per half: pairs within core±m are kept, and bins larger
# than the fixed caps (2048 for the 32 largest "A" bins, 1536 for the 32
# smallest "B" bins) are trimmed to the cap by dropping the farthest
# points — an adaptive per-bin margin that maximizes accuracy at constant
# device cost.  Total rel err ~2.1e-3 vs the 2e-2 gate.
MARGINS = (0.030, 0.022)
CB = 512           # PSUM bank stride in fp32 elements
CH = (CB, CB)      # all chunks are full banks: every ACT read is contiguous
                   # (strided PSUM reads >1792 elements crash HW)
NCHH = (4, 3)      # chunks per bin: A=4 banks, B=3 banks; with the 1-bank
                   # output accumulator that is exactly the 8 PSUM banks,
                   # so the output layer never blocks the next iteration
PH = (CB * NCHH[0], CB * NCHH[1])  # padded points per bin: 2048 / 1536

F32 = mybir.dt.float32
F32R = mybir.dt.float32r  # full-rate fp32 matmul mode on the PE array
F16 = mybir.dt.float16   # output layer: fp16 stationaries double-buffer
                         # their PE loads (f32r reloads serialize, ~2x cost)
TANH = mybir.ActivationFunctionType.Tanh


# ---------------------------------------------------------------- device IR
def build_nc(reps: int = 1, mm_dt=F32R, loop: int = 0, warm: int = 9,
             ysb_bufs: int = 2, l1_split: bool = False, early_out: bool = True,
             abl_layers: int = 3, abl_out: bool = True, abl_dma: bool = True,
             abl_act: bool = True):
    """Build the per-core Bass/Tile program (identical on all 8 cores).

    reps > 1 replays the body with fresh tile allocations for wall-clock
    timing (amortizes launch overhead); loop=N wraps the body in an
    on-device For_i repeating it N times into the same output slot (pure
    compute timing, no per-iteration host transfer).
    """
    nc = bacc.Bacc("TRN2", target_bir_lowering=False, debug=False,
                   num_devices=N_CORES)

    # h0 row r=3g+d: d=0,1 normalized coords, d=2 ones (bias row); per half
    # the first 128 cols of its segment carry w0 (the [12,128] block-diag
    # input weights).
    HTOT = 256 + PH[0] + PH[1]
    h0_d = nc.dram_tensor("h0", [12, HTOT], mm_dt, kind="ExternalInput").ap()
    # wbig cols: w1A|w1B (0:256) + w2A|w2B (256:512) + b1A|b1B|b2A|b2B
    # (512:516).  The output weights travel separately as packed fp16
    # (variant v=4h+j at cols 4v..4v+4, lane g's W_out column at +g).
    WBW = 516
    wbig_d = nc.dram_tensor("wbig", [128, WBW], mm_dt,
                            kind="ExternalInput").ap()
    w3p_d = nc.dram_tensor("w3p", [128, 32], F16, kind="ExternalInput").ap()
    # y[rep, h, p, c]: half h; row p=32j+g => chunk j of subnet lane g
    y_d = nc.dram_tensor("y", [reps, 2, 128, CH[0]], F32,
                         kind="ExternalOutput").ap()

    # Software pipelining (timing builds: loop mode or reps>1): the output
    # layer of body r reads the h3 written by body r-1, so the in-order PE
    # queue never stalls waiting for the current body's last tanh — the
    # next body's layer-1 matmuls run instead.  Needs double-buffered h3
    # and an even number of bodies per For_i iteration so the buffer
    # parity is static.  Single-shot (reps=1, loop=0) stays unpipelined.
    pipe = bool(loop) or reps > 1
    if pipe:
        assert not loop or reps % 2 == 0, "pipelined loop needs even reps"

    with tile.TileContext(nc) as tc:
        with (
            tc.tile_pool(name="const", bufs=1) as cpool,
            tc.tile_pool(name="h", bufs=1) as hpool,
            tc.tile_pool(name="h2", bufs=2 if pipe else 1) as h2pool,
            tc.tile_pool(name="psA", bufs=1, space="PSUM") as psapool,
            tc.tile_pool(name="psB", bufs=1, space="PSUM") as psbpool,
            tc.tile_pool(name="yps", bufs=1, space="PSUM") as ypool,
            tc.tile_pool(name="ysb", bufs=ysb_bufs) as ysbpool,
        ):
            # h0 cols: [w0A(128) | ptsA | w0B(128) | ptsB]
            U0 = 128 + CH[0]  # w0A + chunk-0 points: gates the first matmul
            h0 = cpool.tile([12, HTOT], mm_dt, tag="h0")
            wbig = cpool.tile([128, WBW], mm_dt, tag="wbig")
            w3p = cpool.tile([128, 32], F16, tag="w3p")
            nc.sync.dma_start(h0[:, 0:U0], h0_d[:, 0:U0])
            nc.sync.dma_start(wbig[:, 0:WBW], wbig_d[:, 0:WBW])
            nc.sync.dma_start(h0[:, U0:HTOT], h0_d[:, U0:HTOT])
            nc.sync.dma_start(w3p[:], w3p_d[:])
            # Scatter the packed W_out variants into a zeroed [128,1024]
            # fp16 block: variant v=4h+j lives at cols 128v, nonzero only
            # at 128v+32j..+4 (copies are tiny, far off the critical path).
            w3sb = cpool.tile([128, 1024], F16, tag="w3sb")
            nc.gpsimd.memset(w3sb[:].bitcast(F32), 0.0)
            for h in range(2):
                for j in range(NCHH[h]):
                    v = 4 * h + j
                    nc.vector.tensor_copy(
                        w3sb[:, 128 * v + 32 * j:128 * v + 32 * j + 4],
                        w3p[:, 4 * v:4 * v + 4])
            BOFF = 512
            # Optional PE warm-up (garbage matmuls) — costs serial PE time
            # before the first real matmul, so default off for single-shot.
            if warm:
                scratch = cpool.tile([128, 128], mm_dt, tag="scratch")
                nc.gpsimd.memset(scratch[:].bitcast(F32), 0.0)
                for wi in range(warm):
                    wps = psapool.tile([128, NCHH[0] * CB], F32, tag="psA",
                                       name=f"warm_{wi}")
                    nc.tensor.matmul(wps[0:32, 0:128], lhsT=scratch[:, 0:32],
                                     rhs=scratch[:, 0:128], start=True,
                                     stop=True)
            w0off = (0, 128 + PH[0])
            w0 = [h0[0:12, w0off[h]:w0off[h] + 128] for h in range(2)]
            w1 = [wbig[:, 128 * h:128 * (h + 1)] for h in range(2)]
            w2 = [wbig[:, 256 + 128 * h:256 + 128 * (h + 1)] for h in range(2)]
            w3 = [[w3sb[:, (h * 4 + j) * 128:(h * 4 + j + 1) * 128]
                   for j in range(4)] for h in range(2)]
            b1 = [wbig[:, BOFF + h:BOFF + 1 + h].bitcast(F32) for h in range(2)]
            b2 = [wbig[:, BOFF + 2 + h:BOFF + 3 + h].bitcast(F32) for h in range(2)]

            if pipe:
                # Prologue h3 for the first body's (garbage) output pass —
                # must be written once so Tile accepts the read.
                h3_prev = h2pool.tile([128, PH[0] + PH[1]], F16, tag="h2",
                                      name="h3_prologue")
                nc.gpsimd.memset(h3_prev[:].bitcast(F32), 0.0)

            import contextlib
            loop_cm = tc.For_i(0, loop, 1) if loop else contextlib.nullcontext()
            with loop_cm:
              for rep in range(reps):
                  hs = [hpool.tile([128, PH[0] + PH[1]], mm_dt, tag=f"h{l}",
                                   name=f"h{l}_{rep}")
                        for l in range(2)]
                  hs.append(h2pool.tile([128, PH[0] + PH[1]], F16, tag="h2",
                                        name=f"h2_{rep}"))

                  def emit_output(h3, rep=rep):
                      # Output layer, per half: accumulate the half's chunks
                      # into the dedicated 1-bank PSUM accumulator
                      # (column-shifted W_out variants put chunk j's result
                      # on partitions 32j..32j+3), then one DVE copy + one
                      # DMA.  The layer pools are untouched, so the next
                      # body's matmuls never wait on the output path.
                      for half in range(2 if abl_out else 0):
                          nch = NCHH[half]
                          yps = ypool.tile([128, CB], F32, tag="yps",
                                           name=f"yps_{rep}_{half}")
                          for j in range(nch):
                              nc.tensor.matmul(
                                  yps[:, 0:CB],
                                  lhsT=w3[half][j],
                                  rhs=h3[:, half * PH[0] + CB * j:
                                          half * PH[0] + CB * (j + 1)],
                                  start=(j == 0), stop=(j == nch - 1),
                              )
                          y_sb = ysbpool.tile([128, CB], F32, tag="ysb",
                                              name=f"ysb_{rep}_{half}")
                          nc.vector.tensor_copy(y_sb[:], yps[:])
                          if abl_dma:
                              nc.sync.dma_start(y_d[rep, half][:], y_sb[:])

                  for l in range(abl_layers):
                      src = h0 if l == 0 else hs[l - 1]
                      dst = hs[l]
                      K = 12 if l == 0 else 128
                      w = (w0, w1, w2)[l]
                      b = (None, b1, b2)[l]
                      for half in range(2):
                          nch = NCHH[half]
                          pool = (psapool, psbpool)[half]
                          off = (w0off[half] + 128) if l == 0 \
                              else half * PH[0]
                          doff = half * PH[0]
                          ps = pool.tile([128, nch * CB], F32,
                                         tag=("psA", "psB")[half],
                                         name=f"ps_{rep}_{l}_{half}")
                          for c in range(nch):
                              nc.tensor.matmul(
                                  ps[:, CB * c:CB * (c + 1)],
                                  lhsT=w[half],
                                  rhs=src[0:K,
                                          off + CB * c:off + CB * (c + 1)],
                                  start=True, stop=True,
                              )
                          # One tanh per half-layer; the read is contiguous
                          # full banks (strided PSUM reads >1792 elements
                          # crash HW, contiguous ones are fine).
                          if l == 0 and half == 0 and l1_split:
                              units = ((0, 2), (2, 4))
                          else:
                              units = ((0, nch),)
                          for u0, u1 in units:
                              if abl_act:
                                  dst_out = dst[:, doff + CB * u0:
                                                doff + CB * u1]
                                  ps_in = ps[:, CB * u0:CB * u1]
                                  if b is None:
                                      nc.scalar.activation(dst_out, ps_in,
                                                           TANH)
                                  else:
                                      nc.scalar.activation(dst_out, ps_in,
                                                           TANH,
                                                           bias=b[half])
                      if l == 0 and pipe and early_out:
                          # Pipelined: emit the PREVIOUS body's output layer
                          # here, while the ACT chain works on L1 — the PE
                          # has slack and the y matmuls (reading the old h3)
                          # stay off the inter-body critical path.
                          emit_output(h3_prev)
                          h3_prev = hs[2]
                  if not pipe:
                      emit_output(hs[2])
                  elif not early_out:
                      emit_output(h3_prev)
                      h3_prev = hs[2]
    nc.compile()
    return nc


# ---------------------------------------------------------------- host side
def _window_params(lo_core, hi_core, lo_ext, hi_ext):
    overlap = np.maximum(hi_ext - hi_core, lo_core - lo_ext)
    width = hi_ext - lo_ext
    sfac = 4.0 / (2.0 * overlap * width + 1e-8)
    center = (lo_ext + hi_ext) * 0.5
    hwidth = (hi_ext - lo_ext) * 0.5
    return sfac, center, hwidth


def _bin_points(x, lo_ext, hi_ext):
    """Indices of points within core±margin of each subnet (window weight of
    dropped pairs is a few 1e-3 relative), plus the size-sorted slot
    assignment: the 32 largest bins go to half-A slots (margin MARGINS[0],
    cap PH[0]), the 32 smallest to half-B (MARGINS[1], cap PH[1]).

    Returns (bins, order) with order[core*8 + half*4 + g] = subnet id.
    """
    lo_core = lo_ext + EXT
    hi_core = hi_ext - EXT

    def bins_at(m):
        lo = np.maximum(lo_ext, lo_core - m)
        hi = np.minimum(hi_ext, hi_core + m)
        inb = ((x[None, :, :] >= lo[:, None, :])
               & (x[None, :, :] <= hi[:, None, :])).all(-1)
        return [np.where(inb[s])[0] for s in range(S_TOT)]

    bins_a = bins_at(MARGINS[0])
    bins_b = bins_at(MARGINS[1])
    desc = np.argsort([-len(b) for b in bins_a], kind="stable")
    bins = list(bins_a)
    for rank in range(32, S_TOT):
        bins[desc[rank]] = bins_b[desc[rank]]
    order = np.empty(S_TOT, np.int64)
    for core in range(N_CORES):
        for half in range(2):
            for g in range(4):
                order[core * 8 + half * 4 + g] = desc[half * 32 + core * 4 + g]
    for slot in range(S_TOT):
        s = order[slot]
        cap = PH[(slot // 4) % 2]
        idx = bins[s]
        if len(idx) > cap:
            # Trim to the cap by keeping the points closest to the core box
            # (the dropped ones have the smallest window weight).
            lc, hc = lo_ext[s] + EXT, hi_ext[s] - EXT
            d = np.maximum(lc - x[idx], x[idx] - hc).max(-1)
            bins[s] = idx[np.argsort(d, kind="stable")[:cap]]
            bins[s].sort()
    return bins, order


def _pack_inputs(x, bins, order, lo_core, hi_core, lo_ext, hi_ext,
                 W_in, b_in, W_h, b_h, W_out):
    _, center, hwidth = _window_params(lo_core, hi_core, lo_ext, hi_ext)
    w0off = (0, 128 + PH[0])
    in_maps = []
    for core in range(N_CORES):
        h0 = np.zeros((12, 256 + PH[0] + PH[1]), np.float32)
        wbig = np.zeros((128, 516), np.float32)
        w3p = np.zeros((128, 32), np.float16)
        for half in range(2):
            po = w0off[half] + 128
            for g in range(4):
                s = order[core * SUBS_PER_CORE + half * 4 + g]
                idx = bins[s]
                n = len(idx)
                xn = (x[idx] - center[s]) / hwidth[s]
                h0[3 * g + 0, po:po + n] = xn[:, 0]
                h0[3 * g + 1, po:po + n] = xn[:, 1]
                h0[3 * g + 2, po:po + PH[half]] = 1.0
                gs = slice(32 * g, 32 * g + 32)
                h0[3 * g:3 * g + 2, w0off[half] + 32 * g:w0off[half] + 32 * g + 32] = W_in[s].T
                h0[3 * g + 2, w0off[half] + 32 * g:w0off[half] + 32 * g + 32] = b_in[s]
                wbig[gs, 128 * half + 32 * g:128 * half + 32 * g + 32] = W_h[0, s].T
                wbig[gs, 256 + 128 * half + 32 * g:256 + 128 * half + 32 * g + 32] = W_h[1, s].T
                for j in range(NCHH[half]):
                    w3p[gs, 4 * (half * 4 + j) + g] = W_out[s, 0].astype(np.float16)
                wbig[gs, 512 + half] = b_h[0, s]
                wbig[gs, 514 + half] = b_h[1, s]
        in_maps.append({"h0": h0, "wbig": wbig, "w3p": w3p})
    return in_maps


def _combine(results, x, bins, order, lo_core, hi_core, lo_ext, hi_ext,
             b_out, scale, shift, rep=0):
    sfac, _, _ = _window_params(lo_core, hi_core, lo_ext, hi_ext)
    num = np.zeros(N_PTS, np.float64)
    den = np.zeros(N_PTS, np.float64)
    scale = float(scale)
    shift = float(shift)
    for core in range(N_CORES):
        y = results[core]["y"][rep].astype(np.float64)  # [2, 128, CH[0]]
        for half in range(2):
            C = CH[half]
            for g in range(4):
                s = order[core * SUBS_PER_CORE + half * 4 + g]
                idx = bins[s]
                n = len(idx)
                xs = x[idx].astype(np.float64)
                a = sfac[s] * (xs - lo_core[s])
                bb = sfac[s] * (hi_core[s] - xs)
                w = np.prod(1.0 / (1.0 + np.exp(-a)) / (1.0 + np.exp(-bb)),
                            axis=-1)
                ys = np.empty(n, np.float64)
                for c in range((n + C - 1) // C):
                    lo = c * C
                    hi = min(n, lo + C)
                    ys[lo:hi] = y[half, 32 * c + g, :hi - lo]
                yv = (ys + float(b_out[s, 0])) * scale + shift
                np.add.at(num, idx, w * yv)
                np.add.at(den, idx, w)
    return (num / (den + 1e-8)).astype(np.float32)[:, None]


_NC_CACHE = {}


def _run_device(in_maps):
    if "nc" not in _NC_CACHE:
        _NC_CACHE["nc"] = build_nc()
    res = run_bass_kernel_spmd(_NC_CACHE["nc"], in_maps,
                               list(range(N_CORES)))
    return [{"y": np.asarray(r["y"])} for r in res.results]


def _run_device_subprocess(in_maps):
    """Fallback for the intermittent first-run device crash
    (NRT_EXEC_UNIT_UNRECOVERABLE poisons the in-process jax runtime): rerun
    the device part in a fresh process, which gets a fresh device lease."""
    import os
    import subprocess
    import sys
    import tempfile

    here = os.path.dirname(os.path.abspath(__file__))
    with tempfile.TemporaryDirectory() as td:
        inp, outp = os.path.join(td, "in.npz"), os.path.join(td, "out.npz")
        np.savez(inp, **{f"c{i}_{k}": v for i, m in enumerate(in_maps)
                         for k, v in m.items()})
        code = (
            "import sys, numpy as np\n"
            f"sys.path.insert(0, {here!r})\n"
            "import kernel as K\n"
            f"d = np.load({inp!r})\n"
            "maps = [{k.split('_', 1)[1]: d[k] for k in d.files\n"
            "         if k.startswith(f'c{i}_')} for i in range(K.N_CORES)]\n"
            "ys = K._run_device(maps)\n"
            f"np.savez({outp!r}, **{{f'y{{i}}': r['y'] "
            "for i, r in enumerate(ys)})\n"
        )
        last = None
        for _ in range(3):
            p = subprocess.run([sys.executable, "-c", code],
                               capture_output=True, text=True)
            if p.returncode == 0 and os.path.exists(outp):
                d = np.load(outp)
                return [{"y": d[f"y{i}"]} for i in range(N_CORES)]
            last = p.stderr[-2000:]
        raise RuntimeError(f"device subprocess failed repeatedly: {last}")


def kernel(x, lo_core, hi_core, lo_ext, hi_ext,
           W_in, b_in, W_h, b_h, W_out, b_out, scale, shift):
    x = np.asarray(x, np.float32)
    lo_core = np.asarray(lo_core, np.float32)
    hi_core = np.asarray(hi_core, np.float32)
    lo_ext = np.asarray(lo_ext, np.float32)
    hi_ext = np.asarray(hi_ext, np.float32)
    W_in = np.asarray(W_in, np.float32)
    b_in = np.asarray(b_in, np.float32)
    W_h = np.asarray(W_h, np.float32)
    b_h = np.asarray(b_h, np.float32)
    W_out = np.asarray(W_out, np.float32)
    b_out = np.asarray(b_out, np.float32)

    bins, order = _bin_points(x, lo_ext, hi_ext)
    in_maps = _pack_inputs(x, bins, order, lo_core, hi_core, lo_ext, hi_ext,
                           W_in, b_in, W_h, b_h, W_out)
    try:
        results = _run_device(in_maps)
    except Exception:
        results = _run_device_subprocess(in_maps)
    return _combine(results, x, bins, order, lo_core, hi_core, lo_ext,
                    hi_ext, b_out, scale, shift)


# revision 57
# speedup vs baseline: 1.1712x; 1.1712x over previous
"""FBPINN forward kernel for Trainium2 (8 NeuronCores, SPMD data parallel).

Strategy
--------
The reference evaluates 64 small MLPs (2->32->32->32->1, tanh) on all 65536
points and combines them with compactly-supported sigmoid windows:
    u(x) = sum_s w_s(x) y_s(x) / (sum_s w_s(x) + 1e-8)
The window w_s decays like exp(-266*d) with distance d outside subdomain s's
core cell, so pairs beyond d ~ 0.02 contribute only a few 1e-3 relative.  We
bin points into subdomains whose core cell (dilated by 0.030 / 0.022, then
trimmed to fixed caps keeping the closest points) contains them — ~1.7
subnets per point instead of the dense 64 — run the per-subdomain MLP
batches on the device, and scatter/normalize on the host (rel err ~2.1e-3
vs the 2e-2 gate).

Sharding: 8 subdomains per core (subdomain-parallel); bins are size-sorted:
the 32 largest go to half-A slots (4 full-bank chunks, <=2048 points) and
the 32 smallest to half-B (3 chunks, <=1536); all 8 cores run an identical
program (SPMD).

Device kernel (per core): 8 subnets, two halves of 4, each using
block-diagonal [128,128] float32r stationary weights (4 subnets x 32 hidden
on the partition dim; f32r streams 1 col/cycle when the moving dim >= 256).
Points stream on the free dim in full 512-col PSUM-bank chunks, so every
tanh is ONE contiguous scalar-engine read per half-layer (6 total; strided
PSUM reads >1792 elements crash the HW, contiguous ones don't).  The ACT
engine is the bottleneck (1 col/cycle @ 1.2 GHz over 3 layers), so column
count and ACT instruction count dominate.  The input layer folds its bias
via a constant 1.0 row packed into h0.  PSUM is exactly 8 banks: half-A
tile (4) + half-B tile (3) + a dedicated output accumulator (1), each pool
bufs=1 — ACT(A) frees its tile before PE needs it again, and the output
layer never blocks the next body's matmuls.  The output layer accumulates
each half's chunks into the accumulator bank with column-shifted fp16 W_out
block variants (chunk j's 4 outputs land on partitions 32j..32j+3; fp16
stationaries double-buffer their PE loads, unlike f32r), then one DVE copy
and one DMA per half.  The variants are scattered on-device (memset + 7
tiny DVE copies) from a packed [128,32] fp16 input so the weight DMA stays
small.  Timing builds (loop / reps>1) software-pipeline: body r computes
its 3 tanh layers while the OUTPUT of body r-1 (double-buffered h3) issues
during layer 1, keeping the in-order PE queue off the inter-body critical
path.  Windows, output bias/scale/shift and the final scatter-normalize are
host-side (cheap vectorized numpy).
"""

import numpy as np

import concourse.bass as bass
import concourse.tile as tile
from concourse import bacc, mybir
from concourse.bass_utils import run_bass_kernel_spmd

# ---------------------------------------------------------------- constants
N_PTS = 65536
IN_DIM = 2
HID = 32
S_TOT = 64
N_CORES = 8
SUBS_PER_CORE = 8  # 2 halves x 4 subnets
EXT = 0.0375       # reference's extended-box extension beyond the core cell
# Binning margins per half: pairs within core±m are kept, and bins larger
# than the fixed caps (2048 for the 32 largest "A" bins, 1536 for the 32
# smallest "B" bins) are trimmed to the cap by dropping the farthest
# points — an adaptive per-bin margin that maximizes accuracy at constant
# device cost.  Total rel err ~2.1e-3 vs the 2e-2 gate.
MARGINS = (0.030, 0.022)
CB = 512           # PSUM bank stride in fp32 elements
# Chunk widths: A = 4 chunks x 448 (strided ACT read of exactly 1792
# elements — the proven-safe strided limit; >1792 crashes HW), B = 3 full
# banks (contiguous read).  Caps 1792/1536; trimming A bins from 2048 to
# 1792 costs only ~8e-4 rel err and saves 256 ACT columns per layer.
CH = (448, 512)
NCHH = (4, 3)      # chunk counts: A tile 4 banks, B tile 3 banks; with the
                   # 1-bank output accumulator that is exactly 8 PSUM banks,
                   # so the output layer never blocks the next iteration
PH = (CH[0] * NCHH[0], CH[1] * NCHH[1])  # padded points per bin: 1792/1536

F32 = mybir.dt.float32
F32R = mybir.dt.float32r  # full-rate fp32 matmul mode on the PE array
F16 = mybir.dt.float16   # output layer: fp16 stationaries double-buffer
                         # their PE loads (f32r reloads serialize, ~2x cost)
TANH = mybir.ActivationFunctionType.Tanh


# ---------------------------------------------------------------- device IR
def build_nc(reps: int = 1, mm_dt=F32R, loop: int = 0, warm: int = 9,
             ysb_bufs: int = 2, l1_split: bool = False, early_out: bool = True,
             abl_layers: int = 3, abl_out: bool = True, abl_dma: bool = True,
             abl_act: bool = True):
    """Build the per-core Bass/Tile program (identical on all 8 cores).

    reps > 1 replays the body with fresh tile allocations for wall-clock
    timing (amortizes launch overhead); loop=N wraps the body in an
    on-device For_i repeating it N times into the same output slot (pure
    compute timing, no per-iteration host transfer).
    """
    nc = bacc.Bacc("TRN2", target_bir_lowering=False, debug=False,
                   num_devices=N_CORES)

    # h0 row r=3g+d: d=0,1 normalized coords, d=2 ones (bias row); per half
    # the first 128 cols of its segment carry w0 (the [12,128] block-diag
    # input weights).
    HTOT = 256 + PH[0] + PH[1]
    h0_d = nc.dram_tensor("h0", [12, HTOT], mm_dt, kind="ExternalInput").ap()
    # wbig cols: w1A|w1B (0:256) + w2A|w2B (256:512) + b1A|b1B|b2A|b2B
    # (512:516).  The output weights travel separately as packed fp16
    # (variant v=4h+j at cols 4v..4v+4, lane g's W_out column at +g).
    WBW = 516
    wbig_d = nc.dram_tensor("wbig", [128, WBW], mm_dt,
                            kind="ExternalInput").ap()
    w3p_d = nc.dram_tensor("w3p", [128, 32], F16, kind="ExternalInput").ap()
    # y[rep, h, p, c]: half h; row p=32j+g => chunk j of subnet lane g
    y_d = nc.dram_tensor("y", [reps, 2, 128, CB], F32,
                         kind="ExternalOutput").ap()

    # Software pipelining (timing builds: loop mode or reps>1): the output
    # layer of body r reads the h3 written by body r-1, so the in-order PE
    # queue never stalls waiting for the current body's last tanh — the
    # next body's layer-1 matmuls run instead.  Needs double-buffered h3
    # and an even number of bodies per For_i iteration so the buffer
    # parity is static.  Single-shot (reps=1, loop=0) stays unpipelined.
    pipe = bool(loop) or reps > 1
    if pipe:
        assert not loop or reps % 2 == 0, "pipelined loop needs even reps"

    with tile.TileContext(nc) as tc:
        with (
            tc.tile_pool(name="const", bufs=1) as cpool,
            tc.tile_pool(name="h", bufs=1) as hpool,
            tc.tile_pool(name="h2", bufs=2 if pipe else 1) as h2pool,
            tc.tile_pool(name="psA", bufs=1, space="PSUM") as psapool,
            tc.tile_pool(name="psB", bufs=1, space="PSUM") as psbpool,
            tc.tile_pool(name="yps", bufs=1, space="PSUM") as ypool,
            tc.tile_pool(name="ysb", bufs=ysb_bufs) as ysbpool,
        ):
            # h0 cols: [w0A(128) | ptsA | w0B(128) | ptsB]
            U0 = 128 + CH[0]  # w0A + chunk-0 points: gates the first matmul
            h0 = cpool.tile([12, HTOT], mm_dt, tag="h0")
            wbig = cpool.tile([128, WBW], mm_dt, tag="wbig")
            w3p = cpool.tile([128, 32], F16, tag="w3p")
            nc.sync.dma_start(h0[:, 0:U0], h0_d[:, 0:U0])
            nc.sync.dma_start(wbig[:, 0:WBW], wbig_d[:, 0:WBW])
            nc.sync.dma_start(h0[:, U0:HTOT], h0_d[:, U0:HTOT])
            nc.sync.dma_start(w3p[:], w3p_d[:])
            # Scatter the packed W_out variants into a zeroed [128,1024]
            # fp16 block: variant v=4h+j lives at cols 128v, nonzero only
            # at 128v+32j..+4 (copies are tiny, far off the critical path).
            w3sb = cpool.tile([128, 1024], F16, tag="w3sb")
            nc.gpsimd.memset(w3sb[:].bitcast(F32), 0.0)
            for h in range(2):
                for j in range(NCHH[h]):
                    v = 4 * h + j
                    nc.vector.tensor_copy(
                        w3sb[:, 128 * v + 32 * j:128 * v + 32 * j + 4],
                        w3p[:, 4 * v:4 * v + 4])
            BOFF = 512
            # Optional PE warm-up (garbage matmuls) — costs serial PE time
            # before the first real matmul, so default off for single-shot.
            if warm:
                scratch = cpool.tile([128, 128], mm_dt, tag="scratch")
                nc.gpsimd.memset(scratch[:].bitcast(F32), 0.0)
                for wi in range(warm):
                    wps = psapool.tile([128, NCHH[0] * CB], F32, tag="psA",
                                       name=f"warm_{wi}")
                    nc.tensor.matmul(wps[0:32, 0:128], lhsT=scratch[:, 0:32],
                                     rhs=scratch[:, 0:128], start=True,
                                     stop=True)
            w0off = (0, 128 + PH[0])
            w0 = [h0[0:12, w0off[h]:w0off[h] + 128] for h in range(2)]
            w1 = [wbig[:, 128 * h:128 * (h + 1)] for h in range(2)]
            w2 = [wbig[:, 256 + 128 * h:256 + 128 * (h + 1)] for h in range(2)]
            w3 = [[w3sb[:, (h * 4 + j) * 128:(h * 4 + j + 1) * 128]
                   for j in range(4)] for h in range(2)]
            b1 = [wbig[:, BOFF + h:BOFF + 1 + h].bitcast(F32) for h in range(2)]
            b2 = [wbig[:, BOFF + 2 + h:BOFF + 3 + h].bitcast(F32) for h in range(2)]

            if pipe:
                # Prologue h3 for the first body's (garbage) output pass —
                # must be written once so Tile accepts the read.
                h3_prev = h2pool.tile([128, PH[0] + PH[1]], F16, tag="h2",
                                      name="h3_prologue")
                nc.gpsimd.memset(h3_prev[:].bitcast(F32), 0.0)

            import contextlib
            loop_cm = tc.For_i(0, loop, 1) if loop else contextlib.nullcontext()
            with loop_cm:
              for rep in range(reps):
                  hs = [hpool.tile([128, PH[0] + PH[1]], mm_dt, tag=f"h{l}",
                                   name=f"h{l}_{rep}")
                        for l in range(2)]
                  hs.append(h2pool.tile([128, PH[0] + PH[1]], F16, tag="h2",
                                        name=f"h2_{rep}"))

                  def emit_output(h3, rep=rep):
                      # Output layer, per half: accumulate the half's chunks
                      # into the dedicated 1-bank PSUM accumulator
                      # (column-shifted W_out variants put chunk j's result
                      # on partitions 32j..32j+3), then one DVE copy + one
                      # DMA.  The layer pools are untouched, so the next
                      # body's matmuls never wait on the output path.
                      for half in range(2 if abl_out else 0):
                          nch = NCHH[half]
                          C = CH[half]
                          yps = ypool.tile([128, CB], F32, tag="yps",
                                           name=f"yps_{rep}_{half}")
                          for j in range(nch):
                              nc.tensor.matmul(
                                  yps[:, 0:C],
                                  lhsT=w3[half][j],
                                  rhs=h3[:, half * PH[0] + C * j:
                                          half * PH[0] + C * (j + 1)],
                                  start=(j == 0), stop=(j == nch - 1),
                              )
                          y_sb = ysbpool.tile([128, CB], F32, tag="ysb",
                                              name=f"ysb_{rep}_{half}")
                          nc.vector.tensor_copy(y_sb[:, 0:C], yps[:, 0:C])
                          if abl_dma:
                              nc.sync.dma_start(y_d[rep, half][:, 0:C],
                                                y_sb[:, 0:C])

                  for l in range(abl_layers):
                      src = h0 if l == 0 else hs[l - 1]
                      dst = hs[l]
                      K = 12 if l == 0 else 128
                      w = (w0, w1, w2)[l]
                      b = (None, b1, b2)[l]
                      for half in range(2):
                          nch = NCHH[half]
                          C = CH[half]
                          pool = (psapool, psbpool)[half]
                          off = (w0off[half] + 128) if l == 0 \
                              else half * PH[0]
                          doff = half * PH[0]
                          ps = pool.tile([128, nch * CB], F32,
                                         tag=("psA", "psB")[half],
                                         name=f"ps_{rep}_{l}_{half}")
                          for c in range(nch):
                              nc.tensor.matmul(
                                  ps[:, CB * c:CB * c + C],
                                  lhsT=w[half],
                                  rhs=src[0:K,
                                          off + C * c:off + C * (c + 1)],
                                  start=True, stop=True,
                              )
                          # One tanh per half-layer; strided PSUM reads
                          # above 1792 total elements crash HW (contiguous
                          # ones don't), so A reads 4x448 strided and B
                          # reads 3 full banks contiguously.
                          if l == 0 and half == 0 and l1_split:
                              units = ((0, 2), (2, 4))
                          else:
                              units = ((0, nch),)
                          for u0, u1 in units:
                              if abl_act:
                                  nu = u1 - u0
                                  o = doff + C * u0
                                  if C == CB:
                                      dst_out = dst[:, o:o + nu * C]
                                      ps_in = ps[:, CB * u0:CB * u1]
                                  else:
                                      ps_in = ps[:, CB * u0:CB * u1]\
                                          .rearrange("p (u c) -> p u c",
                                                     c=CB)[:, :, 0:C]
                                      dst_out = dst[:, o:o + nu * C]\
                                          .rearrange("p (u c) -> p u c",
                                                     c=C)
                                  if b is None:
                                      nc.scalar.activation(dst_out, ps_in,
                                                           TANH)
                                  else:
                                      nc.scalar.activation(dst_out, ps_in,
                                                           TANH,
                                                           bias=b[half])
                      if l == 0 and pipe and early_out:
                          # Pipelined: emit the PREVIOUS body's output layer
                          # here, while the ACT chain works on L1 — the PE
                          # has slack and the y matmuls (reading the old h3)
                          # stay off the inter-body critical path.
                          emit_output(h3_prev)
                          h3_prev = hs[2]
                  if not pipe:
                      emit_output(hs[2])
                  elif not early_out:
                      emit_output(h3_prev)
                      h3_prev = hs[2]
    nc.compile()
    return nc


# ---------------------------------------------------------------- host side
def _window_params(lo_core, hi_core, lo_ext, hi_ext):
    overlap = np.maximum(hi_ext - hi_core, lo_core - lo_ext)
    width = hi_ext - lo_ext
    sfac = 4.0 / (2.0 * overlap * width + 1e-8)
    center = (lo_ext + hi_ext) * 0.5
    hwidth = (hi_ext - lo_ext) * 0.5
    return sfac, center, hwidth


def _bin_points(x, lo_ext, hi_ext):
    """Indices of points within core±margin of each subnet (window weight of
    dropped pairs is a few 1e-3 relative), plus the size-sorted slot
    assignment: the 32 largest bins go to half-A slots (margin MARGINS[0],
    cap PH[0]), the 32 smallest to half-B (MARGINS[1], cap PH[1]).

    Returns (bins, order) with order[core*8 + half*4 + g] = subnet id.
    """
    lo_core = lo_ext + EXT
    hi_core = hi_ext - EXT

    def bins_at(m):
        lo = np.maximum(lo_ext, lo_core - m)
        hi = np.minimum(hi_ext, hi_core + m)
        inb = ((x[None, :, :] >= lo[:, None, :])
               & (x[None, :, :] <= hi[:, None, :])).all(-1)
        return [np.where(inb[s])[0] for s in range(S_TOT)]

    bins_a = bins_at(MARGINS[0])
    bins_b = bins_at(MARGINS[1])
    desc = np.argsort([-len(b) for b in bins_a], kind="stable")
    bins = list(bins_a)
    for rank in range(32, S_TOT):
        bins[desc[rank]] = bins_b[desc[rank]]
    order = np.empty(S_TOT, np.int64)
    for core in range(N_CORES):
        for half in range(2):
            for g in range(4):
                order[core * 8 + half * 4 + g] = desc[half * 32 + core * 4 + g]
    for slot in range(S_TOT):
        s = order[slot]
        cap = PH[(slot // 4) % 2]
        idx = bins[s]
        if len(idx) > cap:
            # Trim to the cap by keeping the points closest to the core box
            # (the dropped ones have the smallest window weight).
            lc, hc = lo_ext[s] + EXT, hi_ext[s] - EXT
            d = np.maximum(lc - x[idx], x[idx] - hc).max(-1)
            bins[s] = idx[np.argsort(d, kind="stable")[:cap]]
            bins[s].sort()
    return bins, order


def _pack_inputs(x, bins, order, lo_core, hi_core, lo_ext, hi_ext,
                 W_in, b_in, W_h, b_h, W_out):
    _, center, hwidth = _window_params(lo_core, hi_core, lo_ext, hi_ext)
    w0off = (0, 128 + PH[0])
    in_maps = []
    for core in range(N_CORES):
        h0 = np.zeros((12, 256 + PH[0] + PH[1]), np.float32)
        wbig = np.zeros((128, 516), np.float32)
        w3p = np.zeros((128, 32), np.float16)
        for half in range(2):
            po = w0off[half] + 128
            for g in range(4):
                s = order[core * SUBS_PER_CORE + half * 4 + g]
                idx = bins[s]
                n = len(idx)
                xn = (x[idx] - center[s]) / hwidth[s]
                h0[3 * g + 0, po:po + n] = xn[:, 0]
                h0[3 * g + 1, po:po + n] = xn[:, 1]
                h0[3 * g + 2, po:po + PH[half]] = 1.0
                gs = slice(32 * g, 32 * g + 32)
                h0[3 * g:3 * g + 2, w0off[half] + 32 * g:w0off[half] + 32 * g + 32] = W_in[s].T
                h0[3 * g + 2, w0off[half] + 32 * g:w0off[half] + 32 * g + 32] = b_in[s]
                wbig[gs, 128 * half + 32 * g:128 * half + 32 * g + 32] = W_h[0, s].T
                wbig[gs, 256 + 128 * half + 32 * g:256 + 128 * half + 32 * g + 32] = W_h[1, s].T
                for j in range(NCHH[half]):
                    w3p[gs, 4 * (half * 4 + j) + g] = W_out[s, 0].astype(np.float16)
                wbig[gs, 512 + half] = b_h[0, s]
                wbig[gs, 514 + half] = b_h[1, s]
        in_maps.append({"h0": h0, "wbig": wbig, "w3p": w3p})
    return in_maps


def _combine(results, x, bins, order, lo_core, hi_core, lo_ext, hi_ext,
             b_out, scale, shift, rep=0):
    sfac, _, _ = _window_params(lo_core, hi_core, lo_ext, hi_ext)
    num = np.zeros(N_PTS, np.float64)
    den = np.zeros(N_PTS, np.float64)
    scale = float(scale)
    shift = float(shift)
    for core in range(N_CORES):
        y = results[core]["y"][rep].astype(np.float64)  # [2, 128, CH[0]]
        for half in range(2):
            C = CH[half]
            for g in range(4):
                s = order[core * SUBS_PER_CORE + half * 4 + g]
                idx = bins[s]
                n = len(idx)
                xs = x[idx].astype(np.float64)
                a = sfac[s] * (xs - lo_core[s])
                bb = sfac[s] * (hi_core[s] - xs)
                w = np.prod(1.0 / (1.0 + np.exp(-a)) / (1.0 + np.exp(-bb)),
                            axis=-1)
                ys = np.empty(n, np.float64)
                for c in range((n + C - 1) // C):
                    lo = c * C
                    hi = min(n, lo + C)
                    ys[lo:hi] = y[half, 32 * c + g, :hi - lo]
                yv = (ys + float(b_out[s, 0])) * scale + shift
                np.add.at(num, idx, w * yv)
                np.add.at(den, idx, w)
    return (num / (den + 1e-8)).astype(np.float32)[:, None]


_NC_CACHE = {}


def _run_device(in_maps):
    if "nc" not in _NC_CACHE:
        _NC_CACHE["nc"] = build_nc()
    res = run_bass_kernel_spmd(_NC_CACHE["nc"], in_maps,
                               list(range(N_CORES)))
    return [{"y": np.asarray(r["y"])} for r in res.results]


def _run_device_subprocess(in_maps):
    """Fallback for the intermittent first-run device crash
    (NRT_EXEC_UNIT_UNRECOVERABLE poisons the in-process jax runtime): rerun
    the device part in a fresh process, which gets a fresh device lease."""
    import os
    import subprocess
    import sys
    import tempfile

    here = os.path.dirname(os.path.abspath(__file__))
    with tempfile.TemporaryDirectory() as td:
        inp, outp = os.path.join(td, "in.npz"), os.path.join(td, "out.npz")
        np.savez(inp, **{f"c{i}_{k}": v for i, m in enumerate(in_maps)
                         for k, v in m.items()})
        code = (
            "import sys, numpy as np\n"
            f"sys.path.insert(0, {here!r})\n"
            "import kernel as K\n"
            f"d = np.load({inp!r})\n"
            "maps = [{k.split('_', 1)[1]: d[k] for k in d.files\n"
            "         if k.startswith(f'c{i}_')} for i in range(K.N_CORES)]\n"
            "ys = K._run_device(maps)\n"
            f"np.savez({outp!r}, **{{f'y{{i}}': r['y'] "
            "for i, r in enumerate(ys)})\n"
        )
        last = None
        for _ in range(3):
            p = subprocess.run([sys.executable, "-c", code],
                               capture_output=True, text=True)
            if p.returncode == 0 and os.path.exists(outp):
                d = np.load(outp)
                return [{"y": d[f"y{i}"]} for i in range(N_CORES)]
            last = p.stderr[-2000:]
        raise RuntimeError(f"device subprocess failed repeatedly: {last}")


def kernel(x, lo_core, hi_core, lo_ext, hi_ext,
           W_in, b_in, W_h, b_h, W_out, b_out, scale, shift):
    x = np.asarray(x, np.float32)
    lo_core = np.asarray(lo_core, np.float32)
    hi_core = np.asarray(hi_core, np.float32)
    lo_ext = np.asarray(lo_ext, np.float32)
    hi_ext = np.asarray(hi_ext, np.float32)
    W_in = np.asarray(W_in, np.float32)
    b_in = np.asarray(b_in, np.float32)
    W_h = np.asarray(W_h, np.float32)
    b_h = np.asarray(b_h, np.float32)
    W_out = np.asarray(W_out, np.float32)
    b_out = np.asarray(b_out, np.float32)

    bins, order = _bin_points(x, lo_ext, hi_ext)
    in_maps = _pack_inputs(x, bins, order, lo_core, hi_core, lo_ext, hi_ext,
                           W_in, b_in, W_h, b_h, W_out)
    try:
        results = _run_device(in_maps)
    except Exception:
        results = _run_device_subprocess(in_maps)
    return _combine(results, x, bins, order, lo_core, hi_core, lo_ext,
                    hi_ext, b_out, scale, shift)


# revision 58
# speedup vs baseline: 1.2706x; 1.0849x over previous
"""FBPINN forward kernel for Trainium2 (8 NeuronCores, SPMD data parallel).

Strategy
--------
The reference evaluates 64 small MLPs (2->32->32->32->1, tanh) on all 65536
points and combines them with compactly-supported sigmoid windows:
    u(x) = sum_s w_s(x) y_s(x) / (sum_s w_s(x) + 1e-8)
The window w_s decays like exp(-266*d) with distance d outside subdomain s's
core cell, so pairs beyond d ~ 0.02 contribute only a few 1e-3 relative.  We
bin points into subdomains whose core cell (dilated by 0.030 / 0.022, then
trimmed to fixed caps keeping the closest points) contains them — ~1.7
subnets per point instead of the dense 64 — run the per-subdomain MLP
batches on the device, and scatter/normalize on the host (rel err ~2.1e-3
vs the 2e-2 gate).

Sharding: 8 subdomains per core (subdomain-parallel); bins are size-sorted:
the 32 largest go to half-A slots (4 full-bank chunks, <=2048 points) and
the 32 smallest to half-B (3 chunks, <=1536); all 8 cores run an identical
program (SPMD).

Device kernel (per core): 8 subnets, two halves of 4, each using
block-diagonal [128,128] float32r stationary weights (4 subnets x 32 hidden
on the partition dim; f32r streams 1 col/cycle when the moving dim >= 256).
Points stream on the free dim in full 512-col PSUM-bank chunks, so every
tanh is ONE contiguous scalar-engine read per half-layer (6 total; strided
PSUM reads >1792 elements crash the HW, contiguous ones don't).  The ACT
engine is the bottleneck (1 col/cycle @ 1.2 GHz over 3 layers), so column
count and ACT instruction count dominate.  The input layer folds its bias
via a constant 1.0 row packed into h0.  PSUM is exactly 8 banks: half-A
tile (4) + half-B tile (3) + a dedicated output accumulator (1), each pool
bufs=1 — ACT(A) frees its tile before PE needs it again, and the output
layer never blocks the next body's matmuls.  The output layer accumulates
each half's chunks into the accumulator bank with column-shifted fp16 W_out
block variants (chunk j's 4 outputs land on partitions 32j..32j+3; fp16
stationaries double-buffer their PE loads, unlike f32r), then one DVE copy
and one DMA per half.  The variants are scattered on-device (memset + 7
tiny DVE copies) from a packed [128,32] fp16 input so the weight DMA stays
small.  Timing builds (loop / reps>1) software-pipeline: body r computes
its 3 tanh layers while the OUTPUT of body r-1 (double-buffered h3) issues
during layer 1, keeping the in-order PE queue off the inter-body critical
path.  Windows, output bias/scale/shift and the final scatter-normalize are
host-side (cheap vectorized numpy).
"""

import numpy as np

import concourse.bass as bass
import concourse.tile as tile
from concourse import bacc, mybir
from concourse.bass_utils import run_bass_kernel_spmd

# ---------------------------------------------------------------- constants
N_PTS = 65536
IN_DIM = 2
HID = 32
S_TOT = 64
N_CORES = 8
SUBS_PER_CORE = 8  # 2 halves x 4 subnets
EXT = 0.0375       # reference's extended-box extension beyond the core cell
# Binning margins per half: pairs within core±m are kept, and bins larger
# than the fixed caps (2048 for the 32 largest "A" bins, 1536 for the 32
# smallest "B" bins) are trimmed to the cap by dropping the farthest
# points — an adaptive per-bin margin that maximizes accuracy at constant
# device cost.  Total rel err ~2.1e-3 vs the 2e-2 gate.
MARGINS = (0.030, 0.022)
CB = 512           # PSUM bank stride in fp32 elements
# Chunk widths: A = 4 chunks x 448 (strided ACT read of exactly 1792
# elements — the proven-safe strided limit; >1792 crashes HW), B = 3
# chunks x 480 (strided, 1440).  Caps 1792/1440 with closest-point
# trimming; rel err ~5e-3 vs the 2e-2 gate, and 352 fewer ACT columns
# per layer than the full-bank 2048+1536 layout.
CH = (448, 480)
NCHH = (4, 3)      # chunk counts: A tile 4 banks, B tile 3 banks; with the
                   # 1-bank output accumulator that is exactly 8 PSUM banks,
                   # so the output layer never blocks the next iteration
PH = (CH[0] * NCHH[0], CH[1] * NCHH[1])  # padded points per bin: 1792/1536

F32 = mybir.dt.float32
F32R = mybir.dt.float32r  # full-rate fp32 matmul mode on the PE array
F16 = mybir.dt.float16   # output layer: fp16 stationaries double-buffer
                         # their PE loads (f32r reloads serialize, ~2x cost)
TANH = mybir.ActivationFunctionType.Tanh


# ---------------------------------------------------------------- device IR
def build_nc(reps: int = 1, mm_dt=F32R, loop: int = 0, warm: int = 9,
             ysb_bufs: int = 2, l1_split: bool = False, early_out: bool = True,
             abl_layers: int = 3, abl_out: bool = True, abl_dma: bool = True,
             abl_act: bool = True):
    """Build the per-core Bass/Tile program (identical on all 8 cores).

    reps > 1 replays the body with fresh tile allocations for wall-clock
    timing (amortizes launch overhead); loop=N wraps the body in an
    on-device For_i repeating it N times into the same output slot (pure
    compute timing, no per-iteration host transfer).
    """
    nc = bacc.Bacc("TRN2", target_bir_lowering=False, debug=False,
                   num_devices=N_CORES)

    # h0 row r=3g+d: d=0,1 normalized coords, d=2 ones (bias row); per half
    # the first 128 cols of its segment carry w0 (the [12,128] block-diag
    # input weights).
    HTOT = 256 + PH[0] + PH[1]
    h0_d = nc.dram_tensor("h0", [12, HTOT], mm_dt, kind="ExternalInput").ap()
    # wbig cols: w1A|w1B (0:256) + w2A|w2B (256:512) + b1A|b1B|b2A|b2B
    # (512:516).  The output weights travel separately as packed fp16
    # (variant v=4h+j at cols 4v..4v+4, lane g's W_out column at +g).
    WBW = 516
    wbig_d = nc.dram_tensor("wbig", [128, WBW], mm_dt,
                            kind="ExternalInput").ap()
    w3p_d = nc.dram_tensor("w3p", [128, 32], F16, kind="ExternalInput").ap()
    # y[rep, h, p, c]: half h; row p=32j+g => chunk j of subnet lane g
    y_d = nc.dram_tensor("y", [reps, 2, 128, CB], F32,
                         kind="ExternalOutput").ap()

    # Software pipelining (timing builds: loop mode or reps>1): the output
    # layer of body r reads the h3 written by body r-1, so the in-order PE
    # queue never stalls waiting for the current body's last tanh — the
    # next body's layer-1 matmuls run instead.  Needs double-buffered h3
    # and an even number of bodies per For_i iteration so the buffer
    # parity is static.  Single-shot (reps=1, loop=0) stays unpipelined.
    pipe = bool(loop) or reps > 1
    if pipe:
        assert not loop or reps % 2 == 0, "pipelined loop needs even reps"

    with tile.TileContext(nc) as tc:
        with (
            tc.tile_pool(name="const", bufs=1) as cpool,
            tc.tile_pool(name="h", bufs=1) as hpool,
            tc.tile_pool(name="h2", bufs=2 if pipe else 1) as h2pool,
            tc.tile_pool(name="psA", bufs=1, space="PSUM") as psapool,
            tc.tile_pool(name="psB", bufs=1, space="PSUM") as psbpool,
            tc.tile_pool(name="yps", bufs=1, space="PSUM") as ypool,
            tc.tile_pool(name="ysb", bufs=ysb_bufs) as ysbpool,
        ):
            # h0 cols: [w0A(128) | ptsA | w0B(128) | ptsB]
            U0 = 128 + CH[0]  # w0A + chunk-0 points: gates the first matmul
            h0 = cpool.tile([12, HTOT], mm_dt, tag="h0")
            wbig = cpool.tile([128, WBW], mm_dt, tag="wbig")
            w3p = cpool.tile([128, 32], F16, tag="w3p")
            nc.sync.dma_start(h0[:, 0:U0], h0_d[:, 0:U0])
            nc.sync.dma_start(wbig[:, 0:WBW], wbig_d[:, 0:WBW])
            nc.sync.dma_start(h0[:, U0:HTOT], h0_d[:, U0:HTOT])
            nc.sync.dma_start(w3p[:], w3p_d[:])
            # Scatter the packed W_out variants into a zeroed [128,1024]
            # fp16 block: variant v=4h+j lives at cols 128v, nonzero only
            # at 128v+32j..+4 (copies are tiny, far off the critical path).
            w3sb = cpool.tile([128, 1024], F16, tag="w3sb")
            nc.gpsimd.memset(w3sb[:].bitcast(F32), 0.0)
            for h in range(2):
                for j in range(NCHH[h]):
                    v = 4 * h + j
                    nc.vector.tensor_copy(
                        w3sb[:, 128 * v + 32 * j:128 * v + 32 * j + 4],
                        w3p[:, 4 * v:4 * v + 4])
            BOFF = 512
            # Optional PE warm-up (garbage matmuls) — costs serial PE time
            # before the first real matmul, so default off for single-shot.
            if warm:
                scratch = cpool.tile([128, 128], mm_dt, tag="scratch")
                nc.gpsimd.memset(scratch[:].bitcast(F32), 0.0)
                for wi in range(warm):
                    wps = psapool.tile([128, NCHH[0] * CB], F32, tag="psA",
                                       name=f"warm_{wi}")
                    nc.tensor.matmul(wps[0:32, 0:128], lhsT=scratch[:, 0:32],
                                     rhs=scratch[:, 0:128], start=True,
                                     stop=True)
            w0off = (0, 128 + PH[0])
            w0 = [h0[0:12, w0off[h]:w0off[h] + 128] for h in range(2)]
            w1 = [wbig[:, 128 * h:128 * (h + 1)] for h in range(2)]
            w2 = [wbig[:, 256 + 128 * h:256 + 128 * (h + 1)] for h in range(2)]
            w3 = [[w3sb[:, (h * 4 + j) * 128:(h * 4 + j + 1) * 128]
                   for j in range(4)] for h in range(2)]
            b1 = [wbig[:, BOFF + h:BOFF + 1 + h].bitcast(F32) for h in range(2)]
            b2 = [wbig[:, BOFF + 2 + h:BOFF + 3 + h].bitcast(F32) for h in range(2)]

            if pipe:
                # Prologue h3 for the first body's (garbage) output pass —
                # must be written once so Tile accepts the read.
                h3_prev = h2pool.tile([128, PH[0] + PH[1]], F16, tag="h2",
                                      name="h3_prologue")
                nc.gpsimd.memset(h3_prev[:].bitcast(F32), 0.0)

            import contextlib
            loop_cm = tc.For_i(0, loop, 1) if loop else contextlib.nullcontext()
            with loop_cm:
              for rep in range(reps):
                  hs = [hpool.tile([128, PH[0] + PH[1]], mm_dt, tag=f"h{l}",
                                   name=f"h{l}_{rep}")
                        for l in range(2)]
                  hs.append(h2pool.tile([128, PH[0] + PH[1]], F16, tag="h2",
                                        name=f"h2_{rep}"))

                  def emit_output(h3, rep=rep):
                      # Output layer, per half: accumulate the half's chunks
                      # into the dedicated 1-bank PSUM accumulator
                      # (column-shifted W_out variants put chunk j's result
                      # on partitions 32j..32j+3), then one DVE copy + one
                      # DMA.  The layer pools are untouched, so the next
                      # body's matmuls never wait on the output path.
                      for half in range(2 if abl_out else 0):
                          nch = NCHH[half]
                          C = CH[half]
                          yps = ypool.tile([128, CB], F32, tag="yps",
                                           name=f"yps_{rep}_{half}")
                          for j in range(nch):
                              nc.tensor.matmul(
                                  yps[:, 0:C],
                                  lhsT=w3[half][j],
                                  rhs=h3[:, half * PH[0] + C * j:
                                          half * PH[0] + C * (j + 1)],
                                  start=(j == 0), stop=(j == nch - 1),
                              )
                          y_sb = ysbpool.tile([128, CB], F32, tag="ysb",
                                              name=f"ysb_{rep}_{half}")
                          nc.vector.tensor_copy(y_sb[:, 0:C], yps[:, 0:C])
                          if abl_dma:
                              nc.sync.dma_start(y_d[rep, half][:, 0:C],
                                                y_sb[:, 0:C])

                  for l in range(abl_layers):
                      src = h0 if l == 0 else hs[l - 1]
                      dst = hs[l]
                      K = 12 if l == 0 else 128
                      w = (w0, w1, w2)[l]
                      b = (None, b1, b2)[l]
                      for half in range(2):
                          nch = NCHH[half]
                          C = CH[half]
                          pool = (psapool, psbpool)[half]
                          off = (w0off[half] + 128) if l == 0 \
                              else half * PH[0]
                          doff = half * PH[0]
                          ps = pool.tile([128, nch * CB], F32,
                                         tag=("psA", "psB")[half],
                                         name=f"ps_{rep}_{l}_{half}")
                          for c in range(nch):
                              nc.tensor.matmul(
                                  ps[:, CB * c:CB * c + C],
                                  lhsT=w[half],
                                  rhs=src[0:K,
                                          off + C * c:off + C * (c + 1)],
                                  start=True, stop=True,
                              )
                          # One tanh per half-layer; strided PSUM reads
                          # above 1792 total elements crash HW (contiguous
                          # ones don't), so A reads 4x448 strided and B
                          # reads 3 full banks contiguously.
                          if l == 0 and half == 0 and l1_split:
                              units = ((0, 2), (2, 4))
                          else:
                              units = ((0, nch),)
                          for u0, u1 in units:
                              if abl_act:
                                  nu = u1 - u0
                                  o = doff + C * u0
                                  if C == CB:
                                      dst_out = dst[:, o:o + nu * C]
                                      ps_in = ps[:, CB * u0:CB * u1]
                                  else:
                                      ps_in = ps[:, CB * u0:CB * u1]\
                                          .rearrange("p (u c) -> p u c",
                                                     c=CB)[:, :, 0:C]
                                      dst_out = dst[:, o:o + nu * C]\
                                          .rearrange("p (u c) -> p u c",
                                                     c=C)
                                  if b is None:
                                      nc.scalar.activation(dst_out, ps_in,
                                                           TANH)
                                  else:
                                      nc.scalar.activation(dst_out, ps_in,
                                                           TANH,
                                                           bias=b[half])
                      if l == 0 and pipe and early_out:
                          # Pipelined: emit the PREVIOUS body's output layer
                          # here, while the ACT chain works on L1 — the PE
                          # has slack and the y matmuls (reading the old h3)
                          # stay off the inter-body critical path.
                          emit_output(h3_prev)
                          h3_prev = hs[2]
                  if not pipe:
                      emit_output(hs[2])
                  elif not early_out:
                      emit_output(h3_prev)
                      h3_prev = hs[2]
    nc.compile()
    return nc


# ---------------------------------------------------------------- host side
def _window_params(lo_core, hi_core, lo_ext, hi_ext):
    overlap = np.maximum(hi_ext - hi_core, lo_core - lo_ext)
    width = hi_ext - lo_ext
    sfac = 4.0 / (2.0 * overlap * width + 1e-8)
    center = (lo_ext + hi_ext) * 0.5
    hwidth = (hi_ext - lo_ext) * 0.5
    return sfac, center, hwidth


def _bin_points(x, lo_ext, hi_ext):
    """Indices of points within core±margin of each subnet (window weight of
    dropped pairs is a few 1e-3 relative), plus the size-sorted slot
    assignment: the 32 largest bins go to half-A slots (margin MARGINS[0],
    cap PH[0]), the 32 smallest to half-B (MARGINS[1], cap PH[1]).

    Returns (bins, order) with order[core*8 + half*4 + g] = subnet id.
    """
    lo_core = lo_ext + EXT
    hi_core = hi_ext - EXT

    def bins_at(m):
        lo = np.maximum(lo_ext, lo_core - m)
        hi = np.minimum(hi_ext, hi_core + m)
        inb = ((x[None, :, :] >= lo[:, None, :])
               & (x[None, :, :] <= hi[:, None, :])).all(-1)
        return [np.where(inb[s])[0] for s in range(S_TOT)]

    bins_a = bins_at(MARGINS[0])
    bins_b = bins_at(MARGINS[1])
    desc = np.argsort([-len(b) for b in bins_a], kind="stable")
    bins = list(bins_a)
    for rank in range(32, S_TOT):
        bins[desc[rank]] = bins_b[desc[rank]]
    order = np.empty(S_TOT, np.int64)
    for core in range(N_CORES):
        for half in range(2):
            for g in range(4):
                order[core * 8 + half * 4 + g] = desc[half * 32 + core * 4 + g]
    for slot in range(S_TOT):
        s = order[slot]
        cap = PH[(slot // 4) % 2]
        idx = bins[s]
        if len(idx) > cap:
            # Trim to the cap by keeping the points closest to the core box
            # (the dropped ones have the smallest window weight).
            lc, hc = lo_ext[s] + EXT, hi_ext[s] - EXT
            d = np.maximum(lc - x[idx], x[idx] - hc).max(-1)
            bins[s] = idx[np.argsort(d, kind="stable")[:cap]]
            bins[s].sort()
    return bins, order


def _pack_inputs(x, bins, order, lo_core, hi_core, lo_ext, hi_ext,
                 W_in, b_in, W_h, b_h, W_out):
    _, center, hwidth = _window_params(lo_core, hi_core, lo_ext, hi_ext)
    w0off = (0, 128 + PH[0])
    in_maps = []
    for core in range(N_CORES):
        h0 = np.zeros((12, 256 + PH[0] + PH[1]), np.float32)
        wbig = np.zeros((128, 516), np.float32)
        w3p = np.zeros((128, 32), np.float16)
        for half in range(2):
            po = w0off[half] + 128
            for g in range(4):
                s = order[core * SUBS_PER_CORE + half * 4 + g]
                idx = bins[s]
                n = len(idx)
                xn = (x[idx] - center[s]) / hwidth[s]
                h0[3 * g + 0, po:po + n] = xn[:, 0]
                h0[3 * g + 1, po:po + n] = xn[:, 1]
                h0[3 * g + 2, po:po + PH[half]] = 1.0
                gs = slice(32 * g, 32 * g + 32)
                h0[3 * g:3 * g + 2, w0off[half] + 32 * g:w0off[half] + 32 * g + 32] = W_in[s].T
                h0[3 * g + 2, w0off[half] + 32 * g:w0off[half] + 32 * g + 32] = b_in[s]
                wbig[gs, 128 * half + 32 * g:128 * half + 32 * g + 32] = W_h[0, s].T
                wbig[gs, 256 + 128 * half + 32 * g:256 + 128 * half + 32 * g + 32] = W_h[1, s].T
                for j in range(NCHH[half]):
                    w3p[gs, 4 * (half * 4 + j) + g] = W_out[s, 0].astype(np.float16)
                wbig[gs, 512 + half] = b_h[0, s]
                wbig[gs, 514 + half] = b_h[1, s]
        in_maps.append({"h0": h0, "wbig": wbig, "w3p": w3p})
    return in_maps


def _combine(results, x, bins, order, lo_core, hi_core, lo_ext, hi_ext,
             b_out, scale, shift, rep=0):
    sfac, _, _ = _window_params(lo_core, hi_core, lo_ext, hi_ext)
    num = np.zeros(N_PTS, np.float64)
    den = np.zeros(N_PTS, np.float64)
    scale = float(scale)
    shift = float(shift)
    for core in range(N_CORES):
        y = results[core]["y"][rep].astype(np.float64)  # [2, 128, CH[0]]
        for half in range(2):
            C = CH[half]
            for g in range(4):
                s = order[core * SUBS_PER_CORE + half * 4 + g]
                idx = bins[s]
                n = len(idx)
                xs = x[idx].astype(np.float64)
                a = sfac[s] * (xs - lo_core[s])
                bb = sfac[s] * (hi_core[s] - xs)
                w = np.prod(1.0 / (1.0 + np.exp(-a)) / (1.0 + np.exp(-bb)),
                            axis=-1)
                ys = np.empty(n, np.float64)
                for c in range((n + C - 1) // C):
                    lo = c * C
                    hi = min(n, lo + C)
                    ys[lo:hi] = y[half, 32 * c + g, :hi - lo]
                yv = (ys + float(b_out[s, 0])) * scale + shift
                np.add.at(num, idx, w * yv)
                np.add.at(den, idx, w)
    return (num / (den + 1e-8)).astype(np.float32)[:, None]


_NC_CACHE = {}


def _run_device(in_maps):
    if "nc" not in _NC_CACHE:
        _NC_CACHE["nc"] = build_nc()
    res = run_bass_kernel_spmd(_NC_CACHE["nc"], in_maps,
                               list(range(N_CORES)))
    return [{"y": np.asarray(r["y"])} for r in res.results]


def _run_device_subprocess(in_maps):
    """Fallback for the intermittent first-run device crash
    (NRT_EXEC_UNIT_UNRECOVERABLE poisons the in-process jax runtime): rerun
    the device part in a fresh process, which gets a fresh device lease."""
    import os
    import subprocess
    import sys
    import tempfile

    here = os.path.dirname(os.path.abspath(__file__))
    with tempfile.TemporaryDirectory() as td:
        inp, outp = os.path.join(td, "in.npz"), os.path.join(td, "out.npz")
        np.savez(inp, **{f"c{i}_{k}": v for i, m in enumerate(in_maps)
                         for k, v in m.items()})
        code = (
            "import sys, numpy as np\n"
            f"sys.path.insert(0, {here!r})\n"
            "import kernel as K\n"
            f"d = np.load({inp!r})\n"
            "maps = [{k.split('_', 1)[1]: d[k] for k in d.files\n"
            "         if k.startswith(f'c{i}_')} for i in range(K.N_CORES)]\n"
            "ys = K._run_device(maps)\n"
            f"np.savez({outp!r}, **{{f'y{{i}}': r['y'] "
            "for i, r in enumerate(ys)})\n"
        )
        last = None
        for _ in range(3):
            p = subprocess.run([sys.executable, "-c", code],
                               capture_output=True, text=True)
            if p.returncode == 0 and os.path.exists(outp):
                d = np.load(outp)
                return [{"y": d[f"y{i}"]} for i in range(N_CORES)]
            last = p.stderr[-2000:]
        raise RuntimeError(f"device subprocess failed repeatedly: {last}")


def kernel(x, lo_core, hi_core, lo_ext, hi_ext,
           W_in, b_in, W_h, b_h, W_out, b_out, scale, shift):
    x = np.asarray(x, np.float32)
    lo_core = np.asarray(lo_core, np.float32)
    hi_core = np.asarray(hi_core, np.float32)
    lo_ext = np.asarray(lo_ext, np.float32)
    hi_ext = np.asarray(hi_ext, np.float32)
    W_in = np.asarray(W_in, np.float32)
    b_in = np.asarray(b_in, np.float32)
    W_h = np.asarray(W_h, np.float32)
    b_h = np.asarray(b_h, np.float32)
    W_out = np.asarray(W_out, np.float32)
    b_out = np.asarray(b_out, np.float32)

    bins, order = _bin_points(x, lo_ext, hi_ext)
    in_maps = _pack_inputs(x, bins, order, lo_core, hi_core, lo_ext, hi_ext,
                           W_in, b_in, W_h, b_h, W_out)
    try:
        results = _run_device(in_maps)
    except Exception:
        results = _run_device_subprocess(in_maps)
    return _combine(results, x, bins, order, lo_core, hi_core, lo_ext,
                    hi_ext, b_out, scale, shift)
